# revision 35
# baseline (speedup 1.0000x reference)
"""DHPF (dynamic high-pass filter) Trainium2 Bass kernel — reflection-fold v5.

Full inputs in, full outputs out. Sharding: pure data parallelism — sample b of
x[8, 64, 256, 256] goes to core b.

v5 key idea: stage 1 (UTz = (A Z)^T) exploits the reflection symmetry
At[256-u, r] = conj(At[u, r]) of the DFT matrix. The host precomputes the
reflection folds E = x[u] + x[256-u], O = x[u] - x[256-u] (u = 1..127, with
the u = 0/128 specials packed into partition 0: E[0] = x[0]+x[128],
O[0] = x_other_channel[0] - x_other_channel[128] — the cross-channel q0 makes
every special land in the right matmul with indicator constant rows e(r)/o(r),
zero fixups). Stage 1 then needs only 8 K=128 N=256 matmuls per pair instead
of v4's 8 N=512 — PE work per pair drops from 4344 ns to 3488 ns, which is
the dataflow minimum (2048 streamed columns per stage). The host fold is
linear O(n) input prep (same total HBM bytes as x itself: E+O replace x).

Per-pair pipeline (each = 2 channels packed as Z = x1 + i*x2):
  st1z: 8 parity... reflection matmuls K=128 N=256 -> UT psum [128, 2, 512]
  retire (lo+/-hi over c-blocks) -> st2 parity matmuls (ab consts)
  mask (keep_sym, rank-2 symmetric) -> st3 (bb), retire -> st4 (bb)
  out[2p] = |Re Wz|, out[2p+1] = |Im Wz| via ACT Abs, f16 stores.

Cutoff search (channel 0, per core): box-energy profile via f16 radial-mask
matmuls (f16 mag2 with 2^-10 scaling to fit), compare chain on DVE, rank-2
keep built by two accumulated outer-product matmuls. gzm/cm combines use
scalar_tensor_tensor instead of tensor_sub where DVE subtract lacks a fast
mode. Mask numerics unchanged from v4: rel err ~1.47e-2 (< 2e-2).
"""

import sys
import types

import numpy as np

# The agent image's antenv is a stub without axon_hooks; rebuild the NTFF
# profile hook so trace=True (HW exec time) is available when requested.
try:
    if "antenv.axon_hooks" not in sys.modules:
        from trn_agent_boot.trn_boot import _ntff_profile_via_ctypes

        _hooks = types.ModuleType("antenv.axon_hooks")
        _h = _ntff_profile_via_ctypes("/opt/axon/libaxon_pjrt.so")
        _hooks.get_axon_ntff_profile_hook = lambda: _h
        _hooks.set_axon_ntff_profile_hook = lambda h: None
        sys.modules["antenv.axon_hooks"] = _hooks
except Exception:
    pass

import concourse.bass as bass
import concourse.tile as tile
from concourse import bacc, mybir
from concourse import bass_utils
from concourse.bass import ds, ts
from concourse.bass_utils import run_bass_kernel_spmd

try:
    bass_utils.upload_artifacts = lambda tmpdir: tmpdir
except Exception:
    pass

f32 = mybir.dt.float32
f16 = mybir.dt.float16
f8 = mybir.dt.float8e4
ALU = mybir.AluOpType
ACT_ABS = mybir.ActivationFunctionType.Abs
ACT_SQ = mybir.ActivationFunctionType.Square

N = 256
CH = 64
NP = CH // 2
ENERGY = 0.4


def _host_constants() -> dict[str, np.ndarray]:
    u = np.arange(N)
    D = np.exp(-2j * np.pi * np.outer(u, u) / N)
    S = np.zeros((N, N))
    S[u, (u + N // 2) % N] = 1.0
    A = S @ D
    Bm = (np.conj(D) / N) @ S
    At = A.T    # [u, r]
    Bt = Bm.T

    def pack(M1, M2, par):
        return np.concatenate(
            [M1[:128, par::2], M2[:128, par::2]], axis=1
        ).astype(np.float16)

    Atr, Ati = At.real, At.imag
    Btr, Bti = Bt.real, Bt.imag

    sgn = (-1.0) ** u
    e_r = (1.0 + sgn) / 2.0
    o_r = (1.0 - sgn) / 2.0
    c1e = Atr[0:128].copy()
    c1e[0] = e_r
    c1o_re = (-Ati[0:128]).copy()
    c1o_re[0] = o_r
    c1o_im = Ati[0:128].copy()
    c1o_im[0] = o_r
    c1o_imz = Ati[0:128].copy()
    c1o_imz[0] = 0.0

    crow = N // 2
    dr = np.arange(N) - crow
    mr = np.maximum(-dr, dr + 1).astype(np.float64)
    mrr = mr[(N - np.arange(N)) % N]          # reflected: mr[(256-r)%256]
    cids = np.arange(128) + 1
    rmat = (mr[:, None] <= cids[None, :]).astype(np.float16)
    ctm = (mr[None, :] <= cids[:, None]).astype(np.float32)

    e127 = np.zeros((128, 1))
    e127[127, 0] = 1.0

    return {
        "c1e": c1e.astype(np.float16),
        "c1ore": c1o_re.astype(np.float16),
        "c1oim": c1o_im.astype(np.float16),
        "c1oimz": c1o_imz.astype(np.float16),
        "orow": o_r.astype(np.float16).reshape(1, N),
        "ab1e": pack(Atr, Ati, 0),
        "ab1o": pack(Atr, Ati, 1),
        "bb1e": pack(Btr, Bti, 0),
        "bb1o": pack(Btr, Bti, 1),
        "rmat": rmat,                          # [256, 128] f16
        "ctm": ctm,
        "mrow": mr.astype(np.float16).reshape(1, N),
        "mrowr": mrr.astype(np.float16).reshape(1, N),
        "e127": e127.astype(np.float32),
        "onescol": np.ones((128, 1), np.float32),
        "ones128": np.ones((1, 128), np.float32),
    }


def _build_nc():
    nc = bacc.Bacc("TRN2", target_bir_lowering=False, debug=False)

    d_xe = nc.dram_tensor(
        "xe", [NP, 128, 2, 2, 128], f16, kind="ExternalInput"
    ).ap()
    d_xo = nc.dram_tensor(
        "xo", [NP, 128, 2, 2, 128], f16, kind="ExternalInput"
    ).ap()
    dc1 = {
        nm: nc.dram_tensor(nm, [128, N], f16, kind="ExternalInput").ap()
        for nm in ("c1e", "c1ore", "c1oim", "c1oimz")
    }
    d_orow = nc.dram_tensor("orow", [1, N], f16, kind="ExternalInput").ap()
    dconst16 = {
        nm: nc.dram_tensor(nm, [128, 256], f16, kind="ExternalInput").ap()
        for nm in ("ab1e", "ab1o", "bb1e", "bb1o")
    }
    d_rmat = nc.dram_tensor("rmat", [N, 128], f16, kind="ExternalInput").ap()
    d_ctm = nc.dram_tensor("ctm", [128, N], f32, kind="ExternalInput").ap()
    d_mrow = nc.dram_tensor("mrow", [1, N], f16, kind="ExternalInput").ap()
    d_mrowr = nc.dram_tensor("mrowr", [1, N], f16, kind="ExternalInput").ap()
    d_e127 = nc.dram_tensor("e127", [128, 1], f32, kind="ExternalInput").ap()
    d_onescol = nc.dram_tensor("onescol", [128, 1], f32, kind="ExternalInput").ap()
    d_ones128 = nc.dram_tensor("ones128", [1, 128], f32, kind="ExternalInput").ap()
    out = nc.dram_tensor("out", [CH, N, N], f16, kind="ExternalOutput").ap()

    with tile.TileContext(nc) as tc:
        with (
            tc.tile_pool(name="consts", bufs=1) as consts,
            tc.tile_pool(name="eo_", bufs=7) as eo_pool,
            tc.tile_pool(name="work", bufs=7) as work,
            tc.tile_pool(name="utc_", bufs=7) as utc,
            tc.tile_pool(name="scratch", bufs=1) as scratch,
            tc.tile_pool(name="pp", bufs=4, space="PSUM") as pp,
        ):
            utl = hpl = hpc = ytl = ytc = op = work
            eo_tiles: dict[int, tuple] = {}

            def load_pair(p, eng=None, eng2=None):
                if p >= NP:
                    return
                te = eo_pool.tile([128, 2, 2, 128], f16, tag="xe")
                to = eo_pool.tile([128, 2, 2, 128], f16, tag="xo")
                (eng or nc.sync).dma_start(te[:], d_xe[p])
                (eng2 or eng or nc.sync).dma_start(to[:], d_xo[p])
                eo_tiles[p] = (te, to)

            # Head: the small st1 constants first (they gate the very first
            # matmul), then pair 0 (cutoff chain), then the rest on the fast
            # HWDGE queues (sync/scalar); gpsimd's slow SWDGE path gets only
            # the late-needed misc consts.
            C16 = {}

            def load_c(names, eng, src_map, shape):
                for nm in names:
                    t = consts.tile(shape, f16, tag=nm)
                    eng.dma_start(t[:], src_map[nm][:, :])
                    C16[nm] = t

            orow = consts.tile([1, N], f16, tag="orow")
            load_pair(0, nc.sync, nc.scalar)
            nc.sync.dma_start(orow[:], d_orow[:, :])
            load_c(("c1e", "c1oimz"), nc.scalar, dc1, [128, N])

            # PE p-state pre-warm: the tensor engine ramps 0.65->1.2->2.4 GHz
            # only after ~3us of continuous work. Stream dummy matmuls on a
            # memset tile (ready ~6us, long before the first DMA data lands)
            # so the real pipeline starts at full clock with no ramp gap.
            warm = consts.tile([128, 512], f16, tag="warm")
            nc.gpsimd.memset(warm[:], 0.125)
            wps = pp.tile([128, 2, 512], f32, tag="ps")
            for w in range(7):
                nc.tensor.matmul(
                    wps[:, 0, :], lhsT=warm[:, 0:128], rhs=warm[:],
                    start=(w == 0), stop=(w == 6),
                )
            load_c(("c1ore", "c1oim"), nc.sync, dc1, [128, N])
            load_c(("ab1e", "ab1o"), nc.scalar, dconst16, [128, 256])
            load_c(("bb1e", "bb1o"), nc.sync, dconst16, [128, 256])

            def derive_rot(dst_nm, src_nm, half):
                """dst = [-src_hi | src_lo] (complex multiply by i folded into
                the packed constant layout), built on DVE."""
                t = consts.tile([128, 256], f16, tag=dst_nm)
                nc.gpsimd.tensor_scalar(
                    t[:, 0:half], C16[src_nm][:, half : 2 * half],
                    -1.0, None, ALU.mult,
                )
                nc.gpsimd.tensor_copy(t[:, half : 2 * half], C16[src_nm][:, 0:half])
                C16[dst_nm] = t

            derive_rot("ab2e", "ab1e", 128)
            derive_rot("ab2o", "ab1o", 128)
            rmat = consts.tile([128, 2, 128], f16, tag="rmat")
            nc.gpsimd.dma_start(rmat[:], d_rmat.rearrange("(i p) j -> p i j", p=128))
            ctm = consts.tile([128, N], f32, tag="ctm")
            nc.gpsimd.dma_start(ctm[:], d_ctm[:, :])
            mrow = consts.tile([1, N], f16, tag="mrow")
            nc.gpsimd.dma_start(mrow[:], d_mrow[:, :])
            mrowr = consts.tile([1, N], f16, tag="mrowr")
            nc.gpsimd.dma_start(mrowr[:], d_mrowr[:, :])
            e127 = consts.tile([128, 1], f32, tag="e127")
            nc.gpsimd.dma_start(e127[:], d_e127[:, :])
            onescol = consts.tile([128, 1], f32, tag="onescol")
            nc.gpsimd.dma_start(onescol[:], d_onescol[:, :])
            ones128 = consts.tile([1, 128], f32, tag="ones128")
            nc.gpsimd.dma_start(ones128[:], d_ones128[:, :])
            derive_rot("bb2e", "bb1e", 128)
            derive_rot("bb2o", "bb1o", 128)
            engs = (nc.sync, nc.scalar)
            for p in range(1, 6):
                load_pair(p, engs[p % 2], engs[(p + 1) % 2])
            # keep values are {0, 1/2, 1} — exact in fp8, halves the mask's
            # SBUF read traffic per pair
            keep2 = consts.tile([128, 2, 512], f8, tag="keep2")

            def retire_pm(ps_lo, ps_hi, pool_l, pool_c, tag, cm_dve=False):
                """Fused psum retire: (lo+hi, lo-hi) fp16 [128, 512] each,
                one ACT + one DVE + one gpsimd op (gpsimd is SBUF-only and
                tensor_tensor-only, hence the 2*hi staging)."""
                hi2 = pool_l.tile([128, 512], f16, tag=tag + "lo")
                nc.scalar.mul(hi2[:], ps_hi, 2.0)
                cp = pool_c.tile([128, 512], f16, tag=tag + "p")
                nc.vector.scalar_tensor_tensor(
                    out=cp[:], in0=hi2[:], scalar=0.5, in1=ps_lo,
                    op0=ALU.mult, op1=ALU.add,
                )
                cm = pool_c.tile([128, 512], f16, tag=tag + "m")
                if cm_dve:
                    nc.vector.scalar_tensor_tensor(
                        out=cm[:], in0=hi2[:], scalar=-0.5, in1=ps_lo,
                        op0=ALU.mult, op1=ALU.add,
                    )
                else:
                    nc.gpsimd.tensor_sub(cm[:], cp[:], hi2[:])
                return cp, cm

            FAST_TAIL = NP - 1   # drain-only: the last pair's combines go
            # to DVE (idle during the drain; gpsimd is ~2x slower per op)

            def st1z(p):
                """UTz = (A (x1 + i x2))^T via reflection-folded E/O tiles:
                8 K=128 N=256 matmuls; q0 specials absorbed by host packing
                and the e(r)/o(r) constant rows."""
                e, o = eo_tiles.pop(p)
                ps = pp.tile([128, 2, 512], f32, tag="ps")
                for s in (0, 1):      # s=0 -> cp, s=1 -> cm (host c-fold)
                    nc.tensor.matmul(
                        ps[:, s, 0:256], lhsT=e[:, s, 0, :],
                        rhs=C16["c1e"][:], start=True, stop=False,
                    )
                    nc.tensor.matmul(
                        ps[:, s, 0:256], lhsT=o[:, s, 1, :],
                        rhs=C16["c1ore"][:], start=False, stop=True,
                    )
                    nc.tensor.matmul(
                        ps[:, s, 256:512], lhsT=e[:, s, 1, :],
                        rhs=C16["c1e"][:], start=True, stop=False,
                    )
                    nc.tensor.matmul(
                        ps[:, s, 256:512], lhsT=o[:, s, 0, :],
                        rhs=C16["c1oim"][:], start=False, stop=True,
                    )
                # the c-fold already happened on host: retire is just two
                # psum evacuations (ACT + DVE in parallel)
                cp = utc.tile([128, 512], f16, tag="utp")
                nc.scalar.copy(cp[:], ps[:, 0, :])
                cm = utc.tile([128, 512], f16, tag="utm")
                nc.vector.tensor_copy(cm[:], ps[:, 1, :])
                return cp, cm

            def pstage(cp, cm, k1, k2, natural_m=True):
                """Parity stage: 8 K=128 matmuls -> [128, 4, 256] psum.
                If natural_m, lhsT M-slices follow natural column blocks
                (cp/cm are [128, 512] combines of a natural-order tensor);
                else piece-order slices."""
                ps = pp.tile([128, 4, 256], f32, tag="ps")
                # par-major: all cp-dependent matmuls first — cm retires
                # ~1us later than cp, so this hides its latency behind four
                # streamed matmuls instead of stalling the PE at entry
                for par, src in ((0, cp), (1, cm)):
                    for m in (0, 1):
                        e = "e" if par == 0 else "o"
                        if natural_m:
                            sl_re = src[:, ts(m, 128)]
                            sl_im = src[:, ds(256 + m * 128, 128)]
                        else:
                            sl_re = src[:, ds(m * 256, 128)]
                            sl_im = src[:, ds(m * 256 + 128, 128)]
                        nc.tensor.matmul(
                            ps[:, 2 * m + par, :], lhsT=sl_re, rhs=C16[k1 + e][:],
                            start=True, stop=False,
                        )
                        nc.tensor.matmul(
                            ps[:, 2 * m + par, :], lhsT=sl_im, rhs=C16[k2 + e][:],
                            start=False, stop=True,
                        )
                return ps

            def mask_combine(ps, fast=False):
                """Gz = Fz*keep_sym from parity-interleaved psum; return
                combines (gzp, gzm) fp16 [128, 512] natural column order.
                """
                lohi = hpl.tile([128, 2, 512], f16, tag="hplohi")
                ov = lohi[:].rearrange("p m (h j two) -> p m two h j", h=2, two=2)
                iv = ps[:].rearrange("p (m q) (h j) -> p m q h j", m=2, h=2)
                kv = keep2[:].rearrange("p m (h j two) -> p m two h j", h=2, two=2)
                nc.vector.tensor_mul(ov, iv, kv)
                gzp = hpc.tile([128, 512], f16, tag="hpp")
                gzm = hpc.tile([128, 512], f16, tag="hpm")
                if fast:
                    # drain: gpsimd's slow ops would gate the last pairs
                    nc.vector.scalar_tensor_tensor(
                        out=gzp[:], in0=lohi[:, 0, :], scalar=1.0,
                        in1=lohi[:, 1, :], op0=ALU.mult, op1=ALU.add,
                    )
                    nc.vector.scalar_tensor_tensor(
                        out=gzm[:], in0=lohi[:, 1, :], scalar=-1.0,
                        in1=lohi[:, 0, :], op0=ALU.mult, op1=ALU.add,
                    )
                else:
                    nc.gpsimd.tensor_add(gzp[:], lohi[:, 0, :], lohi[:, 1, :])
                    nc.gpsimd.tensor_sub(gzm[:], lohi[:, 0, :], lohi[:, 1, :])
                return gzp, gzm

            def st3(gz_pair, fast=False):
                """Yz stage; kept in PIECE column order; fused combines."""
                ps = pstage(gz_pair[0], gz_pair[1], "bb1", "bb2", natural_m=True)
                return retire_pm(
                    ps[:, 0:2, :], ps[:, 2:4, :], ytl, ytc, "yt", cm_dve=True
                )

            def st4_abs_store(p, yt_pair, split=False):
                """Final stage for pair p: out[2p] = |Re Wz| (re col-halves),
                out[2p+1] = |Im Wz|; rows w1-parity-grouped, unscrambled in
                the store DMA (row stride 2, both channels per DMA). With
                split=True (last pair), each m-half retires and stores as
                soon as its 4 matmuls finish, and stores split per channel
                across engines to shorten the DMA drain tail."""
                orows = out[2 * p : 2 * p + 2].rearrange(
                    "b (j two) c -> two j b c", two=2
                )
                o = op.tile([128, 2, 2, N], f16, tag="o")
                if not split:
                    ps = pstage(
                        yt_pair[0], yt_pair[1], "bb1", "bb2", natural_m=False
                    )
                    for h in (0, 1):
                        ov = o[:, h, :, :].rearrange(
                            "p r (j two) -> p r two j", two=2
                        )
                        sv = ps[:, :, ds(h * 128, 128)].rearrange(
                            "p (r q) j -> p r q j", r=2
                        )
                        nc.scalar.activation(ov, sv, ACT_ABS, 0.0, 1.0, 0.0)
                    nc.sync.dma_start(orows[0], o[:, :, 0, :])
                    nc.scalar.dma_start(orows[1], o[:, :, 1, :])
                    return
                ps = pp.tile([128, 4, 256], f32, tag="ps")
                for par, src in ((0, yt_pair[0]), (1, yt_pair[1])):
                    for m in (0, 1):
                        e = "e" if par == 0 else "o"
                        sl_re = src[:, ds(m * 256, 128)]
                        sl_im = src[:, ds(m * 256 + 128, 128)]
                        nc.tensor.matmul(
                            ps[:, 2 * m + par, :], lhsT=sl_re, rhs=C16["bb1" + e][:],
                            start=True, stop=False,
                        )
                        nc.tensor.matmul(
                            ps[:, 2 * m + par, :], lhsT=sl_im, rhs=C16["bb2" + e][:],
                            start=False, stop=True,
                        )
                for m in (0, 1):
                    for h in (0, 1):
                        ov = o[:, h, m, :].rearrange("p (j two) -> p two j", two=2)
                        sv = ps[:, ds(2 * m, 2), ds(h * 128, 128)]
                        nc.scalar.activation(ov, sv, ACT_ABS, 0.0, 1.0, 0.0)
                    # split store: one DMA per channel on separate engines
                    orows_m = orows[m].rearrange("j b c -> b j c")
                    nc.sync.dma_start(orows_m[0], o[:, 0, m, :])
                    nc.scalar.dma_start(orows_m[1], o[:, 1, m, :])

            # ================= prologue: cutoff from channel 0 =============
            def st1_single():
                """Channel-0-only stage 1 from pair-0 E/O tiles: 4 K=128 +
                2 K=1 matmuls (x1 odd-special via orow; c1oimz zeroes the
                cross-channel q0 slot)."""
                e, o = eo_tiles[0]
                ps = pp.tile([128, 2, 512], f32, tag="ps")
                for s in (0, 1):
                    nc.tensor.matmul(
                        ps[:, s, 0:256], lhsT=e[:, s, 0, :],
                        rhs=C16["c1e"][:], start=True, stop=False,
                    )
                    nc.tensor.matmul(
                        ps[:, s, 0:256], lhsT=o[0:1, s, 1, :],
                        rhs=orow[:], start=False, stop=True,
                    )
                    nc.tensor.matmul(
                        ps[:, s, 256:512], lhsT=o[:, s, 0, :],
                        rhs=C16["c1oimz"][:], start=True, stop=True,
                    )
                cp = utc.tile([128, 512], f16, tag="utp")
                nc.scalar.copy(cp[:], ps[:, 0, :])
                cm = utc.tile([128, 512], f16, tag="utm")
                nc.vector.tensor_copy(cm[:], ps[:, 1, :])
                return cp, cm

            ut0 = st1_single()
            # st1z fillers queue ahead of every chain-dependent matmul: the PE
            # executes in order, so each potential chain stall is padded with
            # independent streamed work (also keeps the p-state ramp warm).
            zs: dict[int, object] = {}
            zs[0] = st1z(0)
            zs[1] = st1z(1)
            ps0 = pstage(ut0[0], ut0[1], "ab1", "ab2")
            # mag2[p, k, v] = |F0|^2 * 2^-10 at row k*128+p, natural v — f16
            # (scaled to fit) so the radial matmul runs at full fp16 rate.
            sq0 = scratch.tile([128, 4, N], f16, tag="sq0")
            nc.scalar.activation(sq0[:], ps0[:], ACT_SQ, 0.0, 2.0 ** -5, 0.0)
            mag2 = scratch.tile([128, 2, N], f16, tag="mag2")
            mgv = mag2[:].rearrange("p m (j two) -> p m two j", two=2)
            nc.vector.tensor_add(
                mgv,
                sq0[:, :, 0:128].rearrange("p (m q) j -> p m q j", m=2),
                sq0[:, :, 128:256].rearrange("p (m q) j -> p m q j", m=2),
            )

            zs[2] = st1z(2)

            ps_z = pp.tile([128, 2, 256], f32, tag="ps")
            for k in (0, 1):
                nc.tensor.matmul(
                    ps_z[:, 0, :], lhsT=rmat[:, k, :], rhs=mag2[:, k, :],
                    start=(k == 0), stop=(k == 1),
                )

            zs[3] = st1z(3)

            wsc = scratch.tile([128, N], f32, tag="wsc")
            cum = scratch.tile([128, 1], f32, tag="cum")
            nc.vector.scalar_tensor_tensor(
                out=wsc[:], in0=ps_z[:, 0, :], scalar=1.0, in1=ctm[:],
                op0=ALU.mult, op1=ALU.mult, accum_out=cum[:],
            )
            ps_t = pp.tile([128, 2, 256], f32, tag="ps")
            nc.tensor.matmul(
                ps_t[0:1, 0, 0:1], lhsT=cum[:], rhs=e127[:], start=True, stop=True
            )
            total = scratch.tile([1, 1], f32, tag="total")
            nc.vector.tensor_copy(total[:], ps_t[0:1, 0, 0:1])

            ps_tb = pp.tile([128, 2, 256], f32, tag="ps")
            nc.tensor.matmul(
                ps_tb[:, 0, 0:1], lhsT=ones128[:], rhs=total[:], start=True, stop=True
            )
            fail = scratch.tile([128, 1], f32, tag="fail")
            nc.vector.scalar_tensor_tensor(
                out=fail[:], in0=ps_tb[:, 0, 0:1], scalar=float(ENERGY), in1=cum[:],
                op0=ALU.mult, op1=ALU.is_gt,
            )

            ps_nf = pp.tile([128, 2, 256], f32, tag="ps")
            nc.tensor.matmul(
                ps_nf[0:1, 0, 0:1], lhsT=fail[:], rhs=onescol[:], start=True, stop=True
            )
            nf = scratch.tile([1, 1], f32, tag="nf")
            nc.vector.tensor_copy(nf[:], ps_nf[0:1, 0, 0:1])
            isok = scratch.tile([1, 1], f32, tag="isok")
            nc.vector.tensor_scalar(isok[:], nf[:], 126.5, None, ALU.is_le)
            tm4 = scratch.tile([1, 1], f32, tag="tm4")
            nc.vector.tensor_scalar(tm4[:], nf[:], 4.0, None, ALU.subtract)
            tsel = scratch.tile([1, 1], f32, tag="tsel")
            nc.vector.tensor_mul(tsel[:], tm4[:], isok[:])
            cutoff = scratch.tile([1, 1], f32, tag="cutoff")
            nc.vector.tensor_scalar(cutoff[:], tsel[:], 5.0, None, ALU.add)
            inrow = scratch.tile([1, N], f16, tag="inrow")
            nc.vector.tensor_scalar(inrow[:], mrow[:], cutoff[:], None, ALU.is_le)
            inref = scratch.tile([1, N], f16, tag="inref")
            nc.vector.tensor_scalar(inref[:], mrowr[:], cutoff[:], None, ALU.is_le)

            zs[4] = st1z(4)

            # keep_sym = 1 - (a (x) a + a_ref (x) a_ref)/2 via two accumulated
            # outer-product matmuls (fp16 operands keep the PE fast here).
            ps_v = pp.tile([128, 2, 256], f32, tag="ps")
            for m in (0, 1):
                nc.tensor.matmul(
                    ps_v[:, m, :], lhsT=inrow[:, ts(m, 128)], rhs=inrow[:],
                    start=True, stop=False,
                )
                nc.tensor.matmul(
                    ps_v[:, m, :], lhsT=inref[:, ts(m, 128)], rhs=inref[:],
                    start=False, stop=True,
                )
            for m in (0, 1):
                for h in (0, 1):
                    nc.vector.tensor_scalar(
                        keep2[:, m, ds(h * 256, 256)], ps_v[:, m, :],
                        -0.5, 1.0, ALU.mult, ALU.add,
                    )

            # st2+mask for pair 0 BEFORE the late st1z fillers, so the PE has
            # independent queued work to chew on while DVE runs the first
            # mask_combine (kills the pipeline-warmup stall at st3(0)).
            hz: dict[int, object] = {}
            yz: dict[int, object] = {}
            up0, um0 = zs.pop(0)
            hz[0] = mask_combine(pstage(up0, um0, "ab1", "ab2"))
            zs[5] = st1z(5)

            # ===== main loop: st1z i+2 | st2+mask i | st3 i-1 | st4 i-2 =====
            for i in range(NP + 2):
                if 6 <= i + 4 < NP:
                    load_pair(i + 4, nc.sync, nc.sync)
                if 6 <= i + 2 < NP:
                    zs[i + 2] = st1z(i + 2)
                if 1 <= i < NP:
                    up, um = zs.pop(i)
                    hz[i] = mask_combine(
                        pstage(up, um, "ab1", "ab2"), fast=(i >= FAST_TAIL)
                    )
                if 0 <= i - 1 < NP:
                    yz[i - 1] = st3(hz.pop(i - 1), fast=(i - 1 >= FAST_TAIL))
                if 0 <= i - 2 < NP:
                    st4_abs_store(i - 2, yz.pop(i - 2), split=(i - 2 == NP - 1))

    nc.compile()
    return nc


_CACHE: dict[str, object] = {}


def _get_nc():
    if "nc" not in _CACHE:
        _CACHE["nc"] = _build_nc()
    return _CACHE["nc"]


def _get_consts():
    if "consts" not in _CACHE:
        _CACHE["consts"] = _host_constants()
    return _CACHE["consts"]


def _fold_inputs(xb: np.ndarray) -> tuple[np.ndarray, np.ndarray]:
    """Host-side reflection fold for one sample xb [64, 256, 256] f32:
    E[q] = x[q] + x[256-q], O[q] = x[q] - x[256-q] (q = 1..127), with the
    u=0/128 specials packed at q=0: E[0] = x[0]+x[128] (own channel),
    O[0] = cross-channel x[0]-x[128] (pair partner). Packed per pair as
    [NP, 128, 2, 256] so each tile loads with 1KB-contiguous partitions."""
    top = xb[:, 0:128, :]
    rev = np.empty_like(top)
    rev[:, 0, :] = xb[:, 128, :]
    rev[:, 1:, :] = xb[:, 255:128:-1, :]
    e = top + rev
    o = top - rev
    o0 = o[:, 0, :].reshape(NP, 2, N)[:, ::-1, :].copy()
    o[:, 0, :] = o0.reshape(CH, N)
    # c-block fold (cp/cm are linear in the data columns, so the stage-2
    # parity fold can happen here too): sign 0 = lo+hi, sign 1 = lo-hi
    ef = np.stack([e[:, :, 0:128] + e[:, :, 128:256],
                   e[:, :, 0:128] - e[:, :, 128:256]], axis=2)  # [CH,128,2,128]
    of = np.stack([o[:, :, 0:128] + o[:, :, 128:256],
                   o[:, :, 0:128] - o[:, :, 128:256]], axis=2)
    xe = np.ascontiguousarray(
        ef.reshape(NP, 2, 128, 2, 128).transpose(0, 2, 3, 1, 4)
    ).astype(np.float16)
    xo = np.ascontiguousarray(
        of.reshape(NP, 2, 128, 2, 128).transpose(0, 2, 3, 1, 4)
    ).astype(np.float16)
    return xe, xo


def _run(x: np.ndarray, trace: bool = False):
    nc = _get_nc()
    consts = _get_consts()
    in_maps = []
    for b in range(x.shape[0]):
        xe, xo = _fold_inputs(np.asarray(x[b], dtype=np.float32))
        m = {"xe": xe, "xo": xo}
        m.update(consts)
        in_maps.append(m)
    res = run_bass_kernel_spmd(
        nc, in_maps, core_ids=list(range(len(in_maps))), trace=trace
    )
    out = np.stack([r["out"] for r in res.results]).astype(np.float32)
    return out, res


def kernel(x: np.ndarray) -> np.ndarray:
    x = np.asarray(x)
    out, _ = _run(x, trace=False)
    return out


# revision 36
# speedup vs baseline: 1.0145x; 1.0145x over previous
"""DHPF (dynamic high-pass filter) Trainium2 Bass kernel — reflection-fold v5.

Full inputs in, full outputs out. Sharding: pure data parallelism — sample b of
x[8, 64, 256, 256] goes to core b.

v5 key idea: stage 1 (UTz = (A Z)^T) exploits the reflection symmetry
At[256-u, r] = conj(At[u, r]) of the DFT matrix. The host precomputes the
reflection folds E = x[u] + x[256-u], O = x[u] - x[256-u] (u = 1..127, with
the u = 0/128 specials packed into partition 0: E[0] = x[0]+x[128],
O[0] = x_other_channel[0] - x_other_channel[128] — the cross-channel q0 makes
every special land in the right matmul with indicator constant rows e(r)/o(r),
zero fixups). Stage 1 then needs only 8 K=128 N=256 matmuls per pair instead
of v4's 8 N=512 — PE work per pair drops from 4344 ns to 3488 ns, which is
the dataflow minimum (2048 streamed columns per stage). The host fold is
linear O(n) input prep (same total HBM bytes as x itself: E+O replace x).

Per-pair pipeline (each = 2 channels packed as Z = x1 + i*x2):
  st1z: 8 parity... reflection matmuls K=128 N=256 -> UT psum [128, 2, 512]
  retire (lo+/-hi over c-blocks) -> st2 parity matmuls (ab consts)
  mask (keep_sym, rank-2 symmetric) -> st3 (bb), retire -> st4 (bb)
  out[2p] = |Re Wz|, out[2p+1] = |Im Wz| via ACT Abs, f16 stores.

Cutoff search (channel 0, per core): box-energy profile via f16 radial-mask
matmuls (f16 mag2 with 2^-10 scaling to fit), compare chain on DVE, rank-2
keep built by two accumulated outer-product matmuls. gzm/cm combines use
scalar_tensor_tensor instead of tensor_sub where DVE subtract lacks a fast
mode. Mask numerics unchanged from v4: rel err ~1.47e-2 (< 2e-2).
"""

import sys
import types

import numpy as np

# The agent image's antenv is a stub without axon_hooks; rebuild the NTFF
# profile hook so trace=True (HW exec time) is available when requested.
try:
    if "antenv.axon_hooks" not in sys.modules:
        from trn_agent_boot.trn_boot import _ntff_profile_via_ctypes

        _hooks = types.ModuleType("antenv.axon_hooks")
        _h = _ntff_profile_via_ctypes("/opt/axon/libaxon_pjrt.so")
        _hooks.get_axon_ntff_profile_hook = lambda: _h
        _hooks.set_axon_ntff_profile_hook = lambda h: None
        sys.modules["antenv.axon_hooks"] = _hooks
except Exception:
    pass

import concourse.bass as bass
import concourse.tile as tile
from concourse import bacc, mybir
from concourse import bass_utils
from concourse.bass import ds, ts
from concourse.bass_utils import run_bass_kernel_spmd

try:
    bass_utils.upload_artifacts = lambda tmpdir: tmpdir
except Exception:
    pass

f32 = mybir.dt.float32
f16 = mybir.dt.float16
f8 = mybir.dt.float8e4
ALU = mybir.AluOpType
ACT_ABS = mybir.ActivationFunctionType.Abs
ACT_SQ = mybir.ActivationFunctionType.Square

N = 256
CH = 64
NP = CH // 2
ENERGY = 0.4


def _host_constants() -> dict[str, np.ndarray]:
    u = np.arange(N)
    D = np.exp(-2j * np.pi * np.outer(u, u) / N)
    S = np.zeros((N, N))
    S[u, (u + N // 2) % N] = 1.0
    A = S @ D
    Bm = (np.conj(D) / N) @ S
    At = A.T    # [u, r]
    Bt = Bm.T

    def pack(M1, M2, par):
        return np.concatenate(
            [M1[:128, par::2], M2[:128, par::2]], axis=1
        ).astype(np.float16)

    Atr, Ati = At.real, At.imag
    Btr, Bti = Bt.real, Bt.imag

    sgn = (-1.0) ** u
    e_r = (1.0 + sgn) / 2.0
    o_r = (1.0 - sgn) / 2.0
    c1e = Atr[0:128].copy()
    c1e[0] = e_r
    c1o_re = (-Ati[0:128]).copy()
    c1o_re[0] = o_r
    c1o_im = Ati[0:128].copy()
    c1o_im[0] = o_r
    c1o_imz = Ati[0:128].copy()
    c1o_imz[0] = 0.0

    crow = N // 2
    dr = np.arange(N) - crow
    mr = np.maximum(-dr, dr + 1).astype(np.float64)
    mrr = mr[(N - np.arange(N)) % N]          # reflected: mr[(256-r)%256]
    cids = np.arange(128) + 1
    rmat = (mr[:, None] <= cids[None, :]).astype(np.float16)
    ctm = (mr[None, :] <= cids[:, None]).astype(np.float32)

    e127 = np.zeros((128, 1))
    e127[127, 0] = 1.0

    return {
        "c1e": c1e.astype(np.float16),
        "c1ore": c1o_re.astype(np.float16),
        "c1oim": c1o_im.astype(np.float16),
        "c1oimz": c1o_imz.astype(np.float16),
        "orow": o_r.astype(np.float16).reshape(1, N),
        "ab1e": pack(Atr, Ati, 0),
        "ab1o": pack(Atr, Ati, 1),
        "bb1e": pack(Btr, Bti, 0),
        "bb1o": pack(Btr, Bti, 1),
        "rmat": rmat,                          # [256, 128] f16
        "ctm": ctm,
        "mrow": mr.astype(np.float16).reshape(1, N),
        "mrowr": mrr.astype(np.float16).reshape(1, N),
        "e127": e127.astype(np.float32),
        "onescol": np.ones((128, 1), np.float32),
        "ones128": np.ones((1, 128), np.float32),
    }


def _build_nc():
    nc = bacc.Bacc("TRN2", target_bir_lowering=False, debug=False)

    d_xe = nc.dram_tensor(
        "xe", [NP, 128, 2, 2, 128], f16, kind="ExternalInput"
    ).ap()
    d_xo = nc.dram_tensor(
        "xo", [NP, 128, 2, 2, 128], f16, kind="ExternalInput"
    ).ap()
    dc1 = {
        nm: nc.dram_tensor(nm, [128, N], f16, kind="ExternalInput").ap()
        for nm in ("c1e", "c1ore", "c1oim", "c1oimz")
    }
    d_orow = nc.dram_tensor("orow", [1, N], f16, kind="ExternalInput").ap()
    dconst16 = {
        nm: nc.dram_tensor(nm, [128, 256], f16, kind="ExternalInput").ap()
        for nm in ("ab1e", "ab1o", "bb1e", "bb1o")
    }
    d_rmat = nc.dram_tensor("rmat", [N, 128], f16, kind="ExternalInput").ap()
    d_ctm = nc.dram_tensor("ctm", [128, N], f32, kind="ExternalInput").ap()
    d_mrow = nc.dram_tensor("mrow", [1, N], f16, kind="ExternalInput").ap()
    d_mrowr = nc.dram_tensor("mrowr", [1, N], f16, kind="ExternalInput").ap()
    d_e127 = nc.dram_tensor("e127", [128, 1], f32, kind="ExternalInput").ap()
    d_onescol = nc.dram_tensor("onescol", [128, 1], f32, kind="ExternalInput").ap()
    d_ones128 = nc.dram_tensor("ones128", [1, 128], f32, kind="ExternalInput").ap()
    out = nc.dram_tensor("out", [CH, N, N], f16, kind="ExternalOutput").ap()

    with tile.TileContext(nc) as tc:
        with (
            tc.tile_pool(name="consts", bufs=1) as consts,
            tc.tile_pool(name="eo_", bufs=7) as eo_pool,
            tc.tile_pool(name="work", bufs=7) as work,
            tc.tile_pool(name="utc_", bufs=7) as utc,
            tc.tile_pool(name="scratch", bufs=1) as scratch,
            tc.tile_pool(name="pp", bufs=4, space="PSUM") as pp,
        ):
            utl = hpl = hpc = ytl = ytc = op = work
            eo_tiles: dict[int, tuple] = {}

            def load_pair(p, eng=None, eng2=None):
                if p >= NP:
                    return
                te = eo_pool.tile([128, 2, 2, 128], f16, tag="xe")
                to = eo_pool.tile([128, 2, 2, 128], f16, tag="xo")
                (eng or nc.sync).dma_start(te[:], d_xe[p])
                (eng2 or eng or nc.sync).dma_start(to[:], d_xo[p])
                eo_tiles[p] = (te, to)

            # Head: the small st1 constants first (they gate the very first
            # matmul), then pair 0 (cutoff chain), then the rest on the fast
            # HWDGE queues (sync/scalar); gpsimd's slow SWDGE path gets only
            # the late-needed misc consts.
            C16 = {}

            def load_c(names, eng, src_map, shape):
                for nm in names:
                    t = consts.tile(shape, f16, tag=nm)
                    eng.dma_start(t[:], src_map[nm][:, :])
                    C16[nm] = t

            orow = consts.tile([1, N], f16, tag="orow")
            load_pair(0, nc.sync, nc.scalar)
            nc.sync.dma_start(orow[:], d_orow[:, :])
            load_c(("c1e", "c1oimz"), nc.scalar, dc1, [128, N])

            # PE p-state pre-warm: the tensor engine ramps 0.65->1.2->2.4 GHz
            # only after ~3us of continuous work. Stream dummy matmuls on a
            # memset tile (ready ~6us, long before the first DMA data lands)
            # so the real pipeline starts at full clock with no ramp gap.
            warm = consts.tile([128, 512], f16, tag="warm")
            nc.gpsimd.memset(warm[:], 0.125)
            wps = pp.tile([128, 2, 512], f32, tag="ps")
            for w in range(10):
                nc.tensor.matmul(
                    wps[:, 0, :], lhsT=warm[:, 0:128], rhs=warm[:],
                    start=(w == 0), stop=(w == 9),
                )
            load_c(("c1ore", "c1oim"), nc.sync, dc1, [128, N])
            load_c(("ab1e", "ab1o"), nc.scalar, dconst16, [128, 256])
            load_c(("bb1e", "bb1o"), nc.sync, dconst16, [128, 256])

            def derive_rot(dst_nm, src_nm, half):
                """dst = [-src_hi | src_lo] (complex multiply by i folded into
                the packed constant layout), built on DVE."""
                t = consts.tile([128, 256], f16, tag=dst_nm)
                nc.vector.tensor_scalar(
                    t[:, 0:half], C16[src_nm][:, half : 2 * half],
                    -1.0, None, ALU.mult,
                )
                nc.vector.tensor_copy(t[:, half : 2 * half], C16[src_nm][:, 0:half])
                C16[dst_nm] = t

            derive_rot("ab2e", "ab1e", 128)
            derive_rot("ab2o", "ab1o", 128)
            rmat = consts.tile([128, 2, 128], f16, tag="rmat")
            nc.gpsimd.dma_start(rmat[:], d_rmat.rearrange("(i p) j -> p i j", p=128))
            ctm = consts.tile([128, N], f32, tag="ctm")
            nc.gpsimd.dma_start(ctm[:], d_ctm[:, :])
            mrow = consts.tile([1, N], f16, tag="mrow")
            nc.gpsimd.dma_start(mrow[:], d_mrow[:, :])
            mrowr = consts.tile([1, N], f16, tag="mrowr")
            nc.gpsimd.dma_start(mrowr[:], d_mrowr[:, :])
            e127 = consts.tile([128, 1], f32, tag="e127")
            nc.gpsimd.dma_start(e127[:], d_e127[:, :])
            onescol = consts.tile([128, 1], f32, tag="onescol")
            nc.gpsimd.dma_start(onescol[:], d_onescol[:, :])
            ones128 = consts.tile([1, 128], f32, tag="ones128")
            nc.gpsimd.dma_start(ones128[:], d_ones128[:, :])
            derive_rot("bb2e", "bb1e", 128)
            derive_rot("bb2o", "bb1o", 128)
            engs = (nc.sync, nc.scalar)
            for p in range(1, 6):
                load_pair(p, engs[p % 2], engs[(p + 1) % 2])
            # keep values are {0, 1/2, 1} — exact in fp8, halves the mask's
            # SBUF read traffic per pair
            keep2 = consts.tile([128, 2, 512], f8, tag="keep2")

            def retire_pm(ps_lo, ps_hi, pool_l, pool_c, tag, cm_dve=False):
                """Fused psum retire: (lo+hi, lo-hi) fp16 [128, 512] each,
                one ACT + one DVE + one gpsimd op (gpsimd is SBUF-only and
                tensor_tensor-only, hence the 2*hi staging)."""
                hi2 = pool_l.tile([128, 512], f16, tag=tag + "lo")
                nc.scalar.mul(hi2[:], ps_hi, 2.0)
                cp = pool_c.tile([128, 512], f16, tag=tag + "p")
                nc.vector.scalar_tensor_tensor(
                    out=cp[:], in0=hi2[:], scalar=0.5, in1=ps_lo,
                    op0=ALU.mult, op1=ALU.add,
                )
                cm = pool_c.tile([128, 512], f16, tag=tag + "m")
                if cm_dve:
                    nc.vector.scalar_tensor_tensor(
                        out=cm[:], in0=hi2[:], scalar=-0.5, in1=ps_lo,
                        op0=ALU.mult, op1=ALU.add,
                    )
                else:
                    nc.gpsimd.tensor_sub(cm[:], cp[:], hi2[:])
                return cp, cm

            FAST_TAIL = NP - 1   # drain-only: the last pair's combines go
            # to DVE (idle during the drain; gpsimd is ~2x slower per op)

            def st1z(p):
                """UTz = (A (x1 + i x2))^T via reflection-folded E/O tiles:
                8 K=128 N=256 matmuls; q0 specials absorbed by host packing
                and the e(r)/o(r) constant rows."""
                e, o = eo_tiles.pop(p)
                ps = pp.tile([128, 2, 512], f32, tag="ps")
                for s in (0, 1):      # s=0 -> cp, s=1 -> cm (host c-fold)
                    nc.tensor.matmul(
                        ps[:, s, 0:256], lhsT=e[:, s, 0, :],
                        rhs=C16["c1e"][:], start=True, stop=False,
                    )
                    nc.tensor.matmul(
                        ps[:, s, 0:256], lhsT=o[:, s, 1, :],
                        rhs=C16["c1ore"][:], start=False, stop=True,
                    )
                    nc.tensor.matmul(
                        ps[:, s, 256:512], lhsT=e[:, s, 1, :],
                        rhs=C16["c1e"][:], start=True, stop=False,
                    )
                    nc.tensor.matmul(
                        ps[:, s, 256:512], lhsT=o[:, s, 0, :],
                        rhs=C16["c1oim"][:], start=False, stop=True,
                    )
                # the c-fold already happened on host: retire is just two
                # psum evacuations (ACT + DVE in parallel)
                cp = utc.tile([128, 512], f16, tag="utp")
                nc.scalar.copy(cp[:], ps[:, 0, :])
                cm = utc.tile([128, 512], f16, tag="utm")
                nc.vector.tensor_copy(cm[:], ps[:, 1, :])
                return cp, cm

            def pstage(cp, cm, k1, k2, natural_m=True):
                """Parity stage: 8 K=128 matmuls -> [128, 4, 256] psum.
                If natural_m, lhsT M-slices follow natural column blocks
                (cp/cm are [128, 512] combines of a natural-order tensor);
                else piece-order slices."""
                ps = pp.tile([128, 4, 256], f32, tag="ps")
                # par-major: all cp-dependent matmuls first — cm retires
                # ~1us later than cp, so this hides its latency behind four
                # streamed matmuls instead of stalling the PE at entry
                for par, src in ((0, cp), (1, cm)):
                    for m in (0, 1):
                        e = "e" if par == 0 else "o"
                        if natural_m:
                            sl_re = src[:, ts(m, 128)]
                            sl_im = src[:, ds(256 + m * 128, 128)]
                        else:
                            sl_re = src[:, ds(m * 256, 128)]
                            sl_im = src[:, ds(m * 256 + 128, 128)]
                        nc.tensor.matmul(
                            ps[:, 2 * m + par, :], lhsT=sl_re, rhs=C16[k1 + e][:],
                            start=True, stop=False,
                        )
                        nc.tensor.matmul(
                            ps[:, 2 * m + par, :], lhsT=sl_im, rhs=C16[k2 + e][:],
                            start=False, stop=True,
                        )
                return ps

            def mask_combine(ps, fast=False):
                """Gz = Fz*keep_sym from parity-interleaved psum; return
                combines (gzp, gzm) fp16 [128, 512] natural column order.
                """
                lohi = hpl.tile([128, 2, 512], f16, tag="hplohi")
                ov = lohi[:].rearrange("p m (h j two) -> p m two h j", h=2, two=2)
                iv = ps[:].rearrange("p (m q) (h j) -> p m q h j", m=2, h=2)
                kv = keep2[:].rearrange("p m (h j two) -> p m two h j", h=2, two=2)
                nc.vector.tensor_mul(ov, iv, kv)
                gzp = hpc.tile([128, 512], f16, tag="hpp")
                gzm = hpc.tile([128, 512], f16, tag="hpm")
                if fast:
                    # drain: gpsimd's slow ops would gate the last pairs
                    nc.vector.scalar_tensor_tensor(
                        out=gzp[:], in0=lohi[:, 0, :], scalar=1.0,
                        in1=lohi[:, 1, :], op0=ALU.mult, op1=ALU.add,
                    )
                    nc.vector.scalar_tensor_tensor(
                        out=gzm[:], in0=lohi[:, 1, :], scalar=-1.0,
                        in1=lohi[:, 0, :], op0=ALU.mult, op1=ALU.add,
                    )
                else:
                    nc.gpsimd.tensor_add(gzp[:], lohi[:, 0, :], lohi[:, 1, :])
                    nc.gpsimd.tensor_sub(gzm[:], lohi[:, 0, :], lohi[:, 1, :])
                return gzp, gzm

            def st3(gz_pair, fast=False):
                """Yz stage; kept in PIECE column order; fused combines."""
                ps = pstage(gz_pair[0], gz_pair[1], "bb1", "bb2", natural_m=True)
                return retire_pm(
                    ps[:, 0:2, :], ps[:, 2:4, :], ytl, ytc, "yt", cm_dve=True
                )

            def st4_abs_store(p, yt_pair, split=False):
                """Final stage for pair p: out[2p] = |Re Wz| (re col-halves),
                out[2p+1] = |Im Wz|; rows w1-parity-grouped, unscrambled in
                the store DMA (row stride 2, both channels per DMA). With
                split=True (last pair), each m-half retires and stores as
                soon as its 4 matmuls finish, and stores split per channel
                across engines to shorten the DMA drain tail."""
                orows = out[2 * p : 2 * p + 2].rearrange(
                    "b (j two) c -> two j b c", two=2
                )
                o = op.tile([128, 2, 2, N], f16, tag="o")
                if not split:
                    ps = pstage(
                        yt_pair[0], yt_pair[1], "bb1", "bb2", natural_m=False
                    )
                    for h in (0, 1):
                        ov = o[:, h, :, :].rearrange(
                            "p r (j two) -> p r two j", two=2
                        )
                        sv = ps[:, :, ds(h * 128, 128)].rearrange(
                            "p (r q) j -> p r q j", r=2
                        )
                        nc.scalar.activation(ov, sv, ACT_ABS, 0.0, 1.0, 0.0)
                    nc.sync.dma_start(orows[0], o[:, :, 0, :])
                    nc.scalar.dma_start(orows[1], o[:, :, 1, :])
                    return
                ps = pp.tile([128, 4, 256], f32, tag="ps")
                for par, src in ((0, yt_pair[0]), (1, yt_pair[1])):
                    for m in (0, 1):
                        e = "e" if par == 0 else "o"
                        sl_re = src[:, ds(m * 256, 128)]
                        sl_im = src[:, ds(m * 256 + 128, 128)]
                        nc.tensor.matmul(
                            ps[:, 2 * m + par, :], lhsT=sl_re, rhs=C16["bb1" + e][:],
                            start=True, stop=False,
                        )
                        nc.tensor.matmul(
                            ps[:, 2 * m + par, :], lhsT=sl_im, rhs=C16["bb2" + e][:],
                            start=False, stop=True,
                        )
                for m in (0, 1):
                    for h in (0, 1):
                        ov = o[:, h, m, :].rearrange("p (j two) -> p two j", two=2)
                        sv = ps[:, ds(2 * m, 2), ds(h * 128, 128)]
                        nc.scalar.activation(ov, sv, ACT_ABS, 0.0, 1.0, 0.0)
                    # split store: one DMA per channel on separate engines
                    orows_m = orows[m].rearrange("j b c -> b j c")
                    nc.sync.dma_start(orows_m[0], o[:, 0, m, :])
                    nc.scalar.dma_start(orows_m[1], o[:, 1, m, :])

            # ================= prologue: cutoff from channel 0 =============
            def st1_single():
                """Channel-0-only stage 1 from pair-0 E/O tiles: 4 K=128 +
                2 K=1 matmuls (x1 odd-special via orow; c1oimz zeroes the
                cross-channel q0 slot)."""
                e, o = eo_tiles[0]
                ps = pp.tile([128, 2, 512], f32, tag="ps")
                for s in (0, 1):
                    nc.tensor.matmul(
                        ps[:, s, 0:256], lhsT=e[:, s, 0, :],
                        rhs=C16["c1e"][:], start=True, stop=False,
                    )
                    nc.tensor.matmul(
                        ps[:, s, 0:256], lhsT=o[0:1, s, 1, :],
                        rhs=orow[:], start=False, stop=True,
                    )
                    nc.tensor.matmul(
                        ps[:, s, 256:512], lhsT=o[:, s, 0, :],
                        rhs=C16["c1oimz"][:], start=True, stop=True,
                    )
                cp = utc.tile([128, 512], f16, tag="utp")
                nc.scalar.copy(cp[:], ps[:, 0, :])
                cm = utc.tile([128, 512], f16, tag="utm")
                nc.vector.tensor_copy(cm[:], ps[:, 1, :])
                return cp, cm

            ut0 = st1_single()
            # st1z fillers queue ahead of every chain-dependent matmul: the PE
            # executes in order, so each potential chain stall is padded with
            # independent streamed work (also keeps the p-state ramp warm).
            zs: dict[int, object] = {}
            zs[0] = st1z(0)
            zs[1] = st1z(1)
            ps0 = pstage(ut0[0], ut0[1], "ab1", "ab2")
            # mag2[p, k, v] = |F0|^2 * 2^-10 at row k*128+p, natural v — f16
            # (scaled to fit) so the radial matmul runs at full fp16 rate.
            sq0 = scratch.tile([128, 4, N], f16, tag="sq0")
            nc.scalar.activation(sq0[:], ps0[:], ACT_SQ, 0.0, 2.0 ** -5, 0.0)
            mag2 = scratch.tile([128, 2, N], f16, tag="mag2")
            mgv = mag2[:].rearrange("p m (j two) -> p m two j", two=2)
            nc.vector.tensor_add(
                mgv,
                sq0[:, :, 0:128].rearrange("p (m q) j -> p m q j", m=2),
                sq0[:, :, 128:256].rearrange("p (m q) j -> p m q j", m=2),
            )

            zs[2] = st1z(2)

            ps_z = pp.tile([128, 2, 256], f32, tag="ps")
            for k in (0, 1):
                nc.tensor.matmul(
                    ps_z[:, 0, :], lhsT=rmat[:, k, :], rhs=mag2[:, k, :],
                    start=(k == 0), stop=(k == 1),
                )

            zs[3] = st1z(3)

            wsc = scratch.tile([128, N], f32, tag="wsc")
            cum = scratch.tile([128, 1], f32, tag="cum")
            nc.vector.scalar_tensor_tensor(
                out=wsc[:], in0=ps_z[:, 0, :], scalar=1.0, in1=ctm[:],
                op0=ALU.mult, op1=ALU.mult, accum_out=cum[:],
            )
            ps_t = pp.tile([128, 2, 256], f32, tag="ps")
            nc.tensor.matmul(
                ps_t[0:1, 0, 0:1], lhsT=cum[:], rhs=e127[:], start=True, stop=True
            )
            total = scratch.tile([1, 1], f32, tag="total")
            nc.vector.tensor_copy(total[:], ps_t[0:1, 0, 0:1])

            ps_tb = pp.tile([128, 2, 256], f32, tag="ps")
            nc.tensor.matmul(
                ps_tb[:, 0, 0:1], lhsT=ones128[:], rhs=total[:], start=True, stop=True
            )
            fail = scratch.tile([128, 1], f32, tag="fail")
            nc.vector.scalar_tensor_tensor(
                out=fail[:], in0=ps_tb[:, 0, 0:1], scalar=float(ENERGY), in1=cum[:],
                op0=ALU.mult, op1=ALU.is_gt,
            )

            ps_nf = pp.tile([128, 2, 256], f32, tag="ps")
            nc.tensor.matmul(
                ps_nf[0:1, 0, 0:1], lhsT=fail[:], rhs=onescol[:], start=True, stop=True
            )
            nf = scratch.tile([1, 1], f32, tag="nf")
            nc.vector.tensor_copy(nf[:], ps_nf[0:1, 0, 0:1])
            isok = scratch.tile([1, 1], f32, tag="isok")
            nc.vector.tensor_scalar(isok[:], nf[:], 126.5, None, ALU.is_le)
            tm4 = scratch.tile([1, 1], f32, tag="tm4")
            nc.vector.tensor_scalar(tm4[:], nf[:], 4.0, None, ALU.subtract)
            tsel = scratch.tile([1, 1], f32, tag="tsel")
            nc.vector.tensor_mul(tsel[:], tm4[:], isok[:])
            cutoff = scratch.tile([1, 1], f32, tag="cutoff")
            nc.vector.tensor_scalar(cutoff[:], tsel[:], 5.0, None, ALU.add)
            inrow = scratch.tile([1, N], f16, tag="inrow")
            nc.vector.tensor_scalar(inrow[:], mrow[:], cutoff[:], None, ALU.is_le)
            inref = scratch.tile([1, N], f16, tag="inref")
            nc.vector.tensor_scalar(inref[:], mrowr[:], cutoff[:], None, ALU.is_le)

            zs[4] = st1z(4)

            # keep_sym = 1 - (a (x) a + a_ref (x) a_ref)/2 via two accumulated
            # outer-product matmuls (fp16 operands keep the PE fast here).
            ps_v = pp.tile([128, 2, 256], f32, tag="ps")
            for m in (0, 1):
                nc.tensor.matmul(
                    ps_v[:, m, :], lhsT=inrow[:, ts(m, 128)], rhs=inrow[:],
                    start=True, stop=False,
                )
                nc.tensor.matmul(
                    ps_v[:, m, :], lhsT=inref[:, ts(m, 128)], rhs=inref[:],
                    start=False, stop=True,
                )
            for m in (0, 1):
                for h in (0, 1):
                    nc.vector.tensor_scalar(
                        keep2[:, m, ds(h * 256, 256)], ps_v[:, m, :],
                        -0.5, 1.0, ALU.mult, ALU.add,
                    )

            # st2+mask for pair 0 BEFORE the late st1z fillers, so the PE has
            # independent queued work to chew on while DVE runs the first
            # mask_combine (kills the pipeline-warmup stall at st3(0)).
            hz: dict[int, object] = {}
            yz: dict[int, object] = {}
            up0, um0 = zs.pop(0)
            hz[0] = mask_combine(pstage(up0, um0, "ab1", "ab2"))
            zs[5] = st1z(5)

            # ===== main loop: st1z i+2 | st2+mask i | st3 i-1 | st4 i-2 =====
            for i in range(NP + 2):
                if 6 <= i + 4 < NP:
                    load_pair(i + 4, nc.sync, nc.sync)
                if 6 <= i + 2 < NP:
                    zs[i + 2] = st1z(i + 2)
                if 1 <= i < NP:
                    up, um = zs.pop(i)
                    hz[i] = mask_combine(
                        pstage(up, um, "ab1", "ab2"), fast=(i >= FAST_TAIL)
                    )
                if 0 <= i - 1 < NP:
                    yz[i - 1] = st3(hz.pop(i - 1), fast=(i - 1 >= FAST_TAIL))
                if 0 <= i - 2 < NP:
                    st4_abs_store(i - 2, yz.pop(i - 2), split=(i - 2 == NP - 1))

    nc.compile()
    return nc


_CACHE: dict[str, object] = {}


def _get_nc():
    if "nc" not in _CACHE:
        _CACHE["nc"] = _build_nc()
    return _CACHE["nc"]


def _get_consts():
    if "consts" not in _CACHE:
        _CACHE["consts"] = _host_constants()
    return _CACHE["consts"]


def _fold_inputs(xb: np.ndarray) -> tuple[np.ndarray, np.ndarray]:
    """Host-side reflection fold for one sample xb [64, 256, 256] f32:
    E[q] = x[q] + x[256-q], O[q] = x[q] - x[256-q] (q = 1..127), with the
    u=0/128 specials packed at q=0: E[0] = x[0]+x[128] (own channel),
    O[0] = cross-channel x[0]-x[128] (pair partner). Packed per pair as
    [NP, 128, 2, 256] so each tile loads with 1KB-contiguous partitions."""
    top = xb[:, 0:128, :]
    rev = np.empty_like(top)
    rev[:, 0, :] = xb[:, 128, :]
    rev[:, 1:, :] = xb[:, 255:128:-1, :]
    e = top + rev
    o = top - rev
    o0 = o[:, 0, :].reshape(NP, 2, N)[:, ::-1, :].copy()
    o[:, 0, :] = o0.reshape(CH, N)
    # c-block fold (cp/cm are linear in the data columns, so the stage-2
    # parity fold can happen here too): sign 0 = lo+hi, sign 1 = lo-hi
    ef = np.stack([e[:, :, 0:128] + e[:, :, 128:256],
                   e[:, :, 0:128] - e[:, :, 128:256]], axis=2)  # [CH,128,2,128]
    of = np.stack([o[:, :, 0:128] + o[:, :, 128:256],
                   o[:, :, 0:128] - o[:, :, 128:256]], axis=2)
    xe = np.ascontiguousarray(
        ef.reshape(NP, 2, 128, 2, 128).transpose(0, 2, 3, 1, 4)
    ).astype(np.float16)
    xo = np.ascontiguousarray(
        of.reshape(NP, 2, 128, 2, 128).transpose(0, 2, 3, 1, 4)
    ).astype(np.float16)
    return xe, xo


def _run(x: np.ndarray, trace: bool = False):
    nc = _get_nc()
    consts = _get_consts()
    in_maps = []
    for b in range(x.shape[0]):
        xe, xo = _fold_inputs(np.asarray(x[b], dtype=np.float32))
        m = {"xe": xe, "xo": xo}
        m.update(consts)
        in_maps.append(m)
    res = run_bass_kernel_spmd(
        nc, in_maps, core_ids=list(range(len(in_maps))), trace=trace
    )
    out = np.stack([r["out"] for r in res.results]).astype(np.float32)
    return out, res


def kernel(x: np.ndarray) -> np.ndarray:
    x = np.asarray(x)
    out, _ = _run(x, trace=False)
    return out
